# revision 25
# baseline (speedup 1.0000x reference)
"""Trainium2 Bass kernel for nn_GatedCrossAttention.

Computes, for q,k of shape (B=64, D=1024) and weights Wq,Wk (D,D), Wg (D,2D):
    q_proj = q @ Wq.T + bq
    k_proj = k @ Wk.T + bk
    scores[b,i,j]   = q_proj[b,i] * k_proj[b,j]
    gate_pre[b,i,j] = q_proj[b,i] * w1s[j] + t[b,j]
       with w1s = Wg[:, :D].sum(1),  t = k_proj @ W2.T + bg,  W2 = Wg[:, D:]
    out = softmax_j(scores * sigmoid(sigmoid(gate_pre)))

Sharding: pure data parallel, 8 batches per core on 8 NeuronCores.

Per-core device algorithm (per 128-row chunk of one batch's 1024x1024 matrix):
    PE  : gate_pre = K=2 outer-product matmul -> PSUM
    PE  : scores   = K=1 outer-product matmul -> PSUM
    ACT : u = tanh(0.5*gate_pre)          [sigmoid(x) = 0.5 + 0.5*tanh(x/2)]
    DVE : y = M(u) * scores               [custom fused op; M monic cubic]
    ACT : e = exp(a*y), accum z = sum(e)  [a*M(u) ~= sigmoid(sigmoid(.))]
    DVE : out = e * (1/z)
Both ACT functions (tanh, exp) live in the same activation table set
("exp_and_others"), so there are no table switches in the main loop.
Softmax max-subtraction is skipped: exp arguments are bounded (|x| < ~8).
"""

import sys

for _p in ("/opt/trn_rl_repo",):
    if _p not in sys.path:
        sys.path.append(_p)

import numpy as np

B = 64
D = 1024
NCORES = 8
BLOC = B // NCORES  # 8 batches per core

# --- cubic fit:  sigmoid(0.5 + 0.5*u) ~= A3 * (((u + CC0)*u + CC1)*u + CC2) on [-1,1]
# max abs error ~3.6e-5 (Lawson-iterated minimax, fitted offline).
_P0, _P1, _P2, _P3 = (
    0.6224234076915138,
    0.11748147912979392,
    -0.006919796246243861,
    -0.0019515843371938285,
)
A3 = _P3
CC0 = _P2 / _P3
CC1 = _P1 / _P3
CC2 = _P0 / _P3

_CACHE = {}
TRACE = False
LAST_RESULTS = None


def _make_sigmul_op():
    """Custom DVE op:  out = (((Src0 + C0)*Src0 + C1)*Src0 + C2) * Src1.

    Registered in concourse.dve_ops.OPS (the designed extension point) so the
    per-NEFF DVE table generation picks it up."""
    import concourse.dve_ops as dve_ops
    from concourse.dve_ops import DveOp
    from concourse.dve_spec import C0, C1, C2, Spec, Src0, Src1, lower
    from concourse.dve_uop import DveOpSpec

    NAME = "SIGMUL3_GCA"
    for op in dve_ops.OPS:
        if op.name == NAME:
            return op

    def _ref(in0, in1, s0, s1, imm2):
        x = in0.astype(np.float32)
        return ((((x + s0) * x + s1) * x + imm2) * in1).astype(np.float32)

    spec = Spec(
        body=(((Src0 + C0) * Src0 + C1) * Src0 + C2) * Src1,
        reference=_ref,
    )
    opcode = dve_ops._CUSTOM_DVE_ROW_BASE + len(dve_ops.OPS)
    assert opcode < 0x20
    shas = {}
    for ver in ("v3", "v4"):
        tmp = DveOpSpec(
            name=NAME, opcode=opcode, uops=lower(spec, ver=ver), rd1_en=True
        )
        shas[ver] = tmp.sha(ver)
    op = DveOp(NAME, spec, subdim=False, uops_sha=shas)
    dve_ops.OPS.append(op)
    dve_ops._SUB_OPCODE_FOR_NAME[NAME] = opcode
    dve_ops.CUSTOM_DVE_SPECS[NAME] = spec
    return op


def _build():
    import concourse.bacc as bacc
    import concourse.mybir as mybir
    import concourse.tile as tile

    f32 = mybir.dt.float32
    AF = mybir.ActivationFunctionType
    sigmul = _make_sigmul_op()

    nc = bacc.Bacc(
        "TRN2",
        target_bir_lowering=False,
        debug=False,
        num_devices=NCORES,
    )

    # ---- DRAM I/O ----
    qT = nc.dram_tensor("qT", [D, BLOC], f32, kind="ExternalInput")
    kT = nc.dram_tensor("kT", [D, BLOC], f32, kind="ExternalInput")
    WqT = nc.dram_tensor("WqT", [D, D], f32, kind="ExternalInput")
    WkT = nc.dram_tensor("WkT", [D, D], f32, kind="ExternalInput")
    WtT = nc.dram_tensor("WtT", [D, D], f32, kind="ExternalInput")  # (W2 @ Wk).T
    w1s = nc.dram_tensor("w1s", [1, D], f32, kind="ExternalInput")
    bq = nc.dram_tensor("bq", [1, D], f32, kind="ExternalInput")
    bk = nc.dram_tensor("bk", [1, D], f32, kind="ExternalInput")
    bt = nc.dram_tensor("bt", [1, D], f32, kind="ExternalInput")  # bk@W2.T + bg
    out_d = nc.dram_tensor("out", [BLOC, D, D], f32, kind="ExternalOutput")

    NK = D // 128  # 8 contraction chunks

    with tile.TileContext(nc) as tc:
        with (
            tc.tile_pool(name="spool", bufs=1) as spool,
            tc.tile_pool(name="dpool", bufs=1, space="DRAM") as dpool,
        ):
            projs = []
            with (
                tc.tile_pool(name="wpool", bufs=1) as wpool,
                tc.tile_pool(name="wstream", bufs=3) as wstream,
                tc.tile_pool(name="ppool", bufs=3, space="PSUM") as ppool,
            ):
                # ---- load inputs (small; on the PE queue, off the big streams) ----
                qT_sb = wpool.tile([128, NK, BLOC], f32, tag="qT")
                nc.gpsimd.dma_start(qT_sb[:], qT[:].rearrange("(n p) b -> p n b", p=128))
                kT_sb = wpool.tile([128, NK, BLOC], f32, tag="kT")
                nc.gpsimd.dma_start(kT_sb[:], kT[:].rearrange("(n p) b -> p n b", p=128))

                b_sbs = []
                for nm, dram in (("bq", bq), ("bk", bk), ("bt", bt)):
                    b_sb = wpool.tile([1, D], f32, tag=nm)
                    nc.gpsimd.dma_start(b_sb[:], dram[:])
                    b_sbs.append(b_sb)
                bq_sb, bk_sb, bt_sb = b_sbs

                ones1 = wpool.tile([1, BLOC], f32, tag="ones1")
                nc.vector.memset(ones1[:], 1.0)

                # ---- projections: proj = x @ W.T + b  -> (BLOC, D) in SBUF ----
                # weights streamed in 128-row chunks, double-buffered; each
                # weight on its own DMA queue (issued from an idle engine)
                for nm, xT_sb, w_dram, b_sb, dma_eng in (
                    ("qp", qT_sb, WqT, bq_sb, nc.sync),
                    ("kp", kT_sb, WkT, bk_sb, nc.scalar),
                    ("tp", kT_sb, WtT, bt_sb, nc.gpsimd),
                ):
                    ps = ppool.tile([BLOC, D], f32, tag="proj_ps")
                    for kc in range(NK):
                        wch = wstream.tile([128, D], f32, tag="wc" + nm)
                        dma_eng.dma_start(
                            wch[:], w_dram[128 * kc : 128 * kc + 128, :]
                        )
                        for nb in range(2):
                            sl = slice(512 * nb, 512 * nb + 512)
                            nc.tensor.matmul(
                                ps[:, sl],
                                xT_sb[:, kc, :],
                                wch[:, sl],
                                start=(kc == 0),
                                stop=False,
                            )
                    for nb in range(2):
                        sl = slice(512 * nb, 512 * nb + 512)
                        nc.tensor.matmul(
                            ps[:, sl], ones1[:], b_sb[:, sl], start=False, stop=True
                        )
                    p_sb = spool.tile([BLOC, D], f32, tag=nm)
                    nc.vector.tensor_copy(p_sb[:], ps[:])
                    projs.append(p_sb)
            qp_sb, kp_sb, tp_sb = projs

            # k_proj rows staged to DRAM for later partition-broadcast reads
            kp_dram = dpool.tile([BLOC, D], f32, tag="kp_dram")
            nc.sync.dma_start(kp_dram[:], kp_sb[:])

            # ---- bf16 hi/lo splits for the gate matmul (PE streams bf16 at
            # 2x fp32 rate; hi+lo pairs keep ~f32 precision since the PE
            # multiplies bf16 inputs into exact f32 products) ----
            bf16 = mybir.dt.bfloat16
            w1s_sb = spool.tile([1, D], f32, tag="w1s")
            nc.gpsimd.dma_start(w1s_sb[:], w1s[:])

            def hilo(src, nparts, tagb):
                hi = spool.tile([nparts, D], bf16, tag=tagb + "h")
                nc.vector.tensor_copy(hi[:], src[:])
                lo = spool.tile([nparts, D], bf16, tag=tagb + "l")
                nc.vector.tensor_sub(lo[:], src[:], hi[:])
                # roundtrip through DRAM so rows can be re-read as free-dim
                # concats / broadcasts with single big DMAs
                hid = dpool.tile([nparts, D], bf16, tag=tagb + "hd")
                nc.scalar.dma_start(hid[:], hi[:])
                lod = dpool.tile([nparts, D], bf16, tag=tagb + "ld")
                nc.scalar.dma_start(lod[:], lo[:])
                return hid, lod

            qh, ql = hilo(qp_sb, BLOC, "q")
            th, tl = hilo(tp_sb, BLOC, "t")
            wh, wl = hilo(w1s_sb, 1, "w")

            # ---- staging tiles for the gate matmul operands (bf16) ----
            # gate (K=6): lhsT rows [qh,ql,qh,ql,1,1] x rhs [wh,wh,wl,wl,th,tl]
            #   = (qh+ql)*(wh+wl) + th + tl ~= q*w1s + t
            lhs_sb = spool.tile([6, BLOC * D], bf16, tag="lhs")
            grhs_sb = spool.tile([6, BLOC * D], bf16, tag="grhs")
            nc.vector.memset(lhs_sb[:], 1.0)  # partitions 4,5 stay all-ones
            flat = lambda dr: dr[:].rearrange("b f -> (b f)")
            wbc = lambda dr: dr[0:1, :].partition_broadcast(BLOC)
            nc.sync.dma_start(lhs_sb[0:1, :], flat(qh))
            nc.sync.dma_start(lhs_sb[1:2, :], flat(ql))
            nc.sync.dma_start(lhs_sb[2:3, :], flat(qh))
            nc.sync.dma_start(lhs_sb[3:4, :], flat(ql))
            nc.scalar.dma_start(grhs_sb[0:1, :], wbc(wh))
            nc.scalar.dma_start(grhs_sb[1:2, :], wbc(wh))
            nc.scalar.dma_start(grhs_sb[2:3, :], wbc(wl))
            nc.scalar.dma_start(grhs_sb[3:4, :], wbc(wl))
            nc.scalar.dma_start(grhs_sb[4:5, :], flat(th))
            nc.scalar.dma_start(grhs_sb[5:6, :], flat(tl))

            # ---- qaT: per-partition exp scales.  qaT[p, r*BLOC+b] =
            # A3 * q_proj[b, 128r+p], built via PE transposes. ----
            from concourse.masks import make_identity

            ident = spool.tile([128, 128], f32, tag="ident")
            make_identity(nc, ident[:])
            qaT = spool.tile([128, NK * BLOC], f32, tag="qaT")
            with tc.tile_pool(name="tpool", bufs=2, space="PSUM") as tpool:
                for r in range(NK):
                    pst = tpool.tile([128, BLOC], f32, tag="pst")
                    nc.tensor.transpose(
                        pst[:], qp_sb[:, 128 * r : 128 * r + 128], ident[0:BLOC, 0:BLOC]
                    )
                    nc.vector.tensor_scalar_mul(
                        qaT[:, r * BLOC : (r + 1) * BLOC], pst[:], A3
                    )

            # ---- main loop (row-chunk pairs; tanh batched over a pair) ----
            with (
                tc.tile_pool(name="psg", bufs=2, space="PSUM") as psg,
                tc.tile_pool(name="kbpool", bufs=2) as kbpool,
                tc.tile_pool(name="upool", bufs=2) as upool,
                tc.tile_pool(name="mpool", bufs=3) as mpool,
                tc.tile_pool(name="zpool", bufs=4) as zpool,
            ):
                for b in range(BLOC):
                    # broadcast k_proj[b, :] across all 128 partitions, twice
                    # along the free dim (pair-width custom-op operand)
                    # (via DRAM: SBUF-source DMAs reject step-0 partition reads)
                    kb = kbpool.tile([128, 2, D], f32, tag="kb")
                    kbsrc = kp_dram[b : b + 1, :].partition_broadcast(128)
                    nc.gpsimd.dma_start(kb[:, 0:1, :], kbsrc)
                    nc.gpsimd.dma_start(kb[:, 1:2, :], kbsrc)
                    for rp in range(NK // 2):
                        ps_g = psg.tile([128, 2 * D], f32, tag="g")
                        for c in range(2):
                            r = 2 * rp + c
                            rsl = slice(b * D + 128 * r, b * D + 128 * r + 128)
                            for nb in range(2):
                                csl = slice(
                                    b * D + 512 * nb, b * D + 512 * nb + 512
                                )
                                osl = slice(
                                    1024 * c + 512 * nb, 1024 * c + 512 * nb + 512
                                )
                                nc.tensor.matmul(
                                    ps_g[:, osl], lhs_sb[0:6, rsl],
                                    grhs_sb[0:6, csl], start=True, stop=True,
                                )
                        u = upool.tile([128, 2 * D], f32, tag="u")
                        nc.scalar.activation(u[:], ps_g[:], AF.Tanh, scale=0.5)
                        y = upool.tile([128, 2 * D], f32, tag="y")
                        nc.vector._custom_dve(
                            sigmul, out=y[:],
                            in0=u[:],
                            in1=kb[:].rearrange("p a f -> p (a f)"),
                            s0=CC0, s1=CC1, imm2=CC2,
                        )
                        for c in range(2):
                            r = 2 * rp + c
                            e = mpool.tile([128, D], f32, tag="e")
                            z = zpool.tile([128, 1], f32, tag="z")
                            nc.scalar.activation(
                                e[:], y[:, 1024 * c : 1024 * c + 1024], AF.Exp,
                                scale=qaT[:, r * BLOC + b : r * BLOC + b + 1],
                                accum_out=z[:],
                            )
                            rz = zpool.tile([128, 1], f32, tag="rz")
                            nc.vector.reciprocal(rz[:], z[:])
                            o = mpool.tile([128, D], f32, tag="o")
                            nc.vector.tensor_scalar_mul(o[:], e[:], rz[:])
                            (nc.sync if c == 0 else nc.gpsimd).dma_start(
                                out_d[b, 128 * r : 128 * r + 128, :], o[:]
                            )

    nc.compile()
    return nc


def _prep_host(inputs):
    q = np.ascontiguousarray(np.asarray(inputs["q"], dtype=np.float32))
    k = np.ascontiguousarray(np.asarray(inputs["k"], dtype=np.float32))
    Wq = np.asarray(inputs["Wq"], dtype=np.float32)
    Wk = np.asarray(inputs["Wk"], dtype=np.float32)
    Wg = np.asarray(inputs["Wg"], dtype=np.float32)
    bq = np.asarray(inputs["bq"], dtype=np.float32)
    bk = np.asarray(inputs["bk"], dtype=np.float32)
    bg = np.asarray(inputs["bg"], dtype=np.float32)

    W1 = Wg[:, :D]
    W2 = Wg[:, D:]
    WqT = np.ascontiguousarray(Wq.T)
    WkT = np.ascontiguousarray(Wk.T)
    # t = k_proj @ W2.T + bg = k @ (W2 @ Wk).T + (bk @ W2.T + bg)
    WtT = np.ascontiguousarray(Wk.T @ W2.T)
    bt = (bk @ W2.T + bg).astype(np.float32).reshape(1, D)
    w1s = W1.sum(axis=1).astype(np.float32).reshape(1, D)

    shared = {
        "WqT": WqT, "WkT": WkT, "WtT": WtT,
        "w1s": w1s,
        "bq": bq.reshape(1, D).copy(),
        "bk": bk.reshape(1, D).copy(),
        "bt": bt,
    }
    in_maps = []
    for c in range(NCORES):
        sl = slice(c * BLOC, (c + 1) * BLOC)
        m = dict(shared)
        m["qT"] = np.ascontiguousarray(q[sl].T)
        m["kT"] = np.ascontiguousarray(k[sl].T)
        in_maps.append(m)
    return in_maps


def kernel(**inputs) -> np.ndarray:
    global LAST_RESULTS
    from concourse.bass_utils import run_bass_kernel_spmd

    if "nc" not in _CACHE:
        _CACHE["nc"] = _build()
    nc = _CACHE["nc"]

    in_maps = _prep_host(inputs)
    res = run_bass_kernel_spmd(
        nc, in_maps, core_ids=list(range(NCORES)), trace=TRACE
    )
    LAST_RESULTS = res
    out = np.concatenate([res.results[c]["out"] for c in range(NCORES)], axis=0)
    return out
